# revision 18
# baseline (speedup 1.0000x reference)
"""Trainium2 Bass kernel for nn_DenselyCnnAttLayer.

Reference computation (B=64, S=512, L=6, D=512):
    X = stack([x0..x5], axis=2)                  # [B,S,L,D]
    s = X.sum(-1)                                # [B,S,L]
    logits = einsum('bsl,slm->bsm', s, Ws)       # [B,S,L]
    a = softmax(logits, -1)
    out = einsum('bsl,bsld->bsd', a, X)          # [B,S,D]

Strategy: data-parallel over batch across 8 cores (8 batches/core).
Per core the 4096 (b,s) rows are processed as 32 blocks of 128
partitions with D=512 on the free dim; blocks are loaded in pairs so
each input DMA moves 512 KB.  Row sums split between ScalarE
(activation-with-accum) and VectorE (tensor_scalar-with-accum); softmax
and the weighted accumulation (scalar_tensor_tensor chain) run on
VectorE.  Block stages are software-pipelined two deep — S1(i) row
sums, S2/S3(i-1) logits+exp, S4(i-2) weighted chain + store — so the
in-order engine streams never stall on each other's latest results.
Loads ride the SP HWDGE ring, stores the ACT ring, so store triggers
can't head-of-line block load triggers.
"""

import sys

if "/opt/trn_rl_repo" not in sys.path:
    sys.path.insert(0, "/opt/trn_rl_repo")

import numpy as np

import concourse.bass as bass
import concourse.bacc as bacc
import concourse.mybir as mybir
from concourse import tile
from concourse.bass_utils import run_bass_kernel_spmd

B, S, L, D = 64, 512, 6, 512
N_CORES = 8
B_PER = B // N_CORES       # 8 batches per core
ROWS = B_PER * S           # 4096 rows per core
P = 128                    # SBUF partitions
N_BLOCKS = ROWS // P       # 32 row blocks per core
S_BLOCKS = S // P          # 4 position blocks (Ws varies with position)
KP = 2                     # row blocks per load group (512 KB per DMA)

FP32 = mybir.dt.float32
AF = mybir.ActivationFunctionType
ALU = mybir.AluOpType
AX = mybir.AxisListType

N_DVE_SUMS = 2  # how many of the L row-sums run on VectorE instead of ScalarE


def build_module() -> bass.Bass:
    nc = bacc.Bacc("TRN2", debug=False, num_devices=N_CORES)
    xs = [
        nc.dram_tensor(f"x{j}", [ROWS, D], FP32, kind="ExternalInput").ap()
        for j in range(L)
    ]
    ws = nc.dram_tensor("Ws", [S, L * L], FP32, kind="ExternalInput").ap()
    out = nc.dram_tensor("out", [ROWS, D], FP32, kind="ExternalOutput").ap()

    def group_view(ap, b0, kp):
        # [kp*P, D] DRAM slice -> [P, kp, D] (partition, block, feature)
        return ap[b0 * P : (b0 + kp) * P, :].rearrange("(k p) d -> p k d", p=P)

    # Load groups: pairs in steady state, singles for the last four blocks so
    # the end-of-stream compute drain is one block deep, not two.
    groups = [(b, KP) for b in range(0, N_BLOCKS - 4, KP)]
    groups += [(b, 1) for b in range(N_BLOCKS - 4, N_BLOCKS)]

    with tile.TileContext(nc) as tc:
        with (
            tc.tile_pool(name="wpool", bufs=1) as wpool,
            tc.tile_pool(name="xpool", bufs=4) as xpool,
            tc.tile_pool(name="opool", bufs=4) as opool,
            tc.tile_pool(name="accpool", bufs=4) as accpool,
            tc.tile_pool(name="small", bufs=8) as small,
            tc.tile_pool(name="trashpool", bufs=1) as trashpool,
        ):
            # Kick off group 0's big x loads first so the DMA engines ramp
            # immediately; the tiny Ws loads follow.
            first_x = []
            for j in range(L):
                xt0 = xpool.tile([P, KP, D], FP32, tag=f"x{j}", name=f"x{j}_g0")
                nc.sync.dma_start(out=xt0[:, :, :], in_=group_view(xs[j], 0, KP))
                first_x.append(xt0)

            # Per-position 6x6 matrices, resident for the whole kernel.
            # ws_tiles[blk][p, l*6+m] = Ws[blk*128+p, l, m]
            ws_tiles = []
            for blk in range(S_BLOCKS):
                wt = wpool.tile([P, L * L], FP32, tag=f"ws{blk}")
                nc.sync.dma_start(out=wt[:, :], in_=ws[blk * P : (blk + 1) * P, :])
                ws_tiles.append(wt)

            # Garbage destinations for the row-sum accum trick (never read).
            # Separate tiles per engine so ACT/DVE don't serialize on WAW.
            trash = trashpool.tile([P, D], FP32)
            trash_dve = trashpool.tile([P, D], FP32)

            class Blk:
                __slots__ = ("x", "s", "logits", "e", "sum_e", "b")

            def stage1(st: Blk):
                # Row sums: s[p, j] = sum_d x_j[p, d]
                st.s = small.tile([P, L], FP32, tag="s", name=f"s_{st.b}")
                for j in range(L - N_DVE_SUMS):
                    nc.scalar.activation(
                        trash[:, :], st.x[j], AF.Copy,
                        accum_out=st.s[:, j : j + 1],
                    )
                for j in range(L - N_DVE_SUMS, L):
                    nc.vector.tensor_scalar(
                        out=trash_dve[:, :], in0=st.x[j],
                        scalar1=1.0, scalar2=0.0,
                        op0=ALU.mult, op1=ALU.add,
                        accum_out=st.s[:, j : j + 1],
                    )

            def stage23(st: Blk):
                # logits[p, m] = sum_l s[p, l] * Ws[pos(p), l, m], then exp.
                # No max-subtraction: |logits| < ~20 here, exp is fp32-safe.
                blk = st.b % S_BLOCKS
                prod = small.tile([P, L * L], FP32, tag="prod", name=f"pr_{st.b}")
                nc.vector.tensor_tensor(
                    out=prod[:, :].rearrange("p (l m) -> p l m", m=L),
                    in0=st.s[:, :].unsqueeze(2).broadcast_to((P, L, L)),
                    in1=ws_tiles[blk][:, :].rearrange("p (l m) -> p l m", m=L),
                    op=ALU.mult,
                )
                st.logits = small.tile([P, L], FP32, tag="lg", name=f"lg_{st.b}")
                nc.vector.tensor_reduce(
                    out=st.logits[:, :],
                    in_=prod[:, :].rearrange("p (l m) -> p m l", m=L),
                    axis=AX.X,
                    op=ALU.add,
                )
                st.e = small.tile([P, L], FP32, tag="e", name=f"e_{st.b}")
                st.sum_e = small.tile([P, 1], FP32, tag="se", name=f"se_{st.b}")
                nc.scalar.activation(
                    st.e[:, :], st.logits[:, :], AF.Exp,
                    accum_out=st.sum_e[:, 0:1],
                )

            def stage4(st: Blk):
                # a = e / sum_e ; out[p, d] = sum_j a[p, j] * x_j[p, d]
                recip = small.tile([P, 1], FP32, tag="rc", name=f"rc_{st.b}")
                nc.vector.reciprocal(recip[:, :], st.sum_e[:, :])
                a_t = small.tile([P, L], FP32, tag="a", name=f"a_{st.b}")
                nc.vector.tensor_scalar_mul(a_t[:, :], st.e[:, :], recip[:, 0:1])

                o_t = opool.tile([P, D], FP32, tag="o", name=f"o_{st.b}")
                acc = accpool.tile([P, D], FP32, tag="acc", name=f"ac_{st.b}")
                nc.vector.tensor_scalar_mul(acc[:, :], st.x[0], a_t[:, 0:1])
                bufs = [acc[:, :], o_t[:, :]]
                for j in range(1, L):
                    nc.vector.scalar_tensor_tensor(
                        out=bufs[j % 2],
                        in0=st.x[j],
                        scalar=a_t[:, j : j + 1],
                        in1=bufs[(j + 1) % 2],
                        op0=ALU.mult,
                        op1=ALU.add,
                    )
                # L-1 = 5 steps -> result lands in o_t; store on the ACT ring.
                nc.scalar.dma_start(
                    out=out[st.b * P : (st.b + 1) * P, :], in_=o_t[:, :]
                )

            # Two-deep software pipeline over blocks.
            pending: list[Blk] = []

            def tick():
                if len(pending) >= 2:
                    stage23(pending[-2])
                if len(pending) >= 3:
                    stage4(pending[-3])

            for gi, (b0, kp) in enumerate(groups):
                if gi == 0:
                    xg = first_x
                else:
                    xg = []
                    for j in range(L):
                        xt = xpool.tile(
                            [P, kp, D], FP32, tag=f"x{j}", name=f"x{j}_g{gi}"
                        )
                        nc.sync.dma_start(
                            out=xt[:, :, :], in_=group_view(xs[j], b0, kp)
                        )
                        xg.append(xt)
                for k in range(kp):
                    st = Blk()
                    st.b = b0 + k
                    st.x = [xg[j][:, k, :] for j in range(L)]
                    pending.append(st)
                    stage1(st)
                    tick()
            # flush
            stage23(pending[-1])
            stage4(pending[-2])
            stage4(pending[-1])

    # Legalize for TRN2 (≤1 sync wait per instruction) + register alloc.
    nc.compile()
    return nc


_MODULE_CACHE: bass.Bass | None = None


def _get_module() -> bass.Bass:
    global _MODULE_CACHE
    if _MODULE_CACHE is None:
        _MODULE_CACHE = build_module()
    return _MODULE_CACHE


def make_in_maps(inputs: dict) -> list:
    ws = np.ascontiguousarray(np.asarray(inputs["Ws"], dtype=np.float32)).reshape(
        S, L * L
    )
    in_maps = []
    for c in range(N_CORES):
        m = {
            f"x{j}": np.ascontiguousarray(
                np.asarray(inputs[f"x{j}"], dtype=np.float32)[
                    c * B_PER : (c + 1) * B_PER
                ]
            ).reshape(ROWS, D)
            for j in range(L)
        }
        m["Ws"] = ws
        in_maps.append(m)
    return in_maps


def kernel(**inputs) -> np.ndarray:
    nc = _get_module()
    in_maps = make_in_maps(inputs)
    res = run_bass_kernel_spmd(nc, in_maps, core_ids=list(range(N_CORES)))
    outs = [res.results[c]["out"].reshape(B_PER, S, D) for c in range(N_CORES)]
    return np.concatenate(outs, axis=0)


# revision 24
# speedup vs baseline: 38.4683x; 38.4683x over previous
"""Trainium2 Bass kernel for nn_DenselyCnnAttLayer.

Reference computation (B=64, S=512, L=6, D=512):
    X = stack([x0..x5], axis=2)                  # [B,S,L,D]
    s = X.sum(-1)                                # [B,S,L]
    logits = einsum('bsl,slm->bsm', s, Ws)       # [B,S,L]
    a = softmax(logits, -1)
    out = einsum('bsl,bsld->bsd', a, X)          # [B,S,D]

Strategy: data-parallel over batch across 8 cores (8 batches/core).
Per core the 4096 (b,s) rows are processed as 32 blocks of 128
partitions with D=512 on the free dim; blocks are loaded in pairs so
each input DMA moves 512 KB.  Row sums split between ScalarE
(activation-with-accum) and VectorE (tensor_scalar-with-accum); softmax
and the weighted accumulation (scalar_tensor_tensor chain) run on
VectorE.  Block stages are software-pipelined two deep — S1(i) row
sums, S2/S3(i-1) logits+exp, S4(i-2) weighted chain + store — so the
in-order engine streams never stall on each other's latest results.
Loads ride the SP HWDGE ring, stores the ACT ring, so store triggers
can't head-of-line block load triggers.
"""

import sys

if "/opt/trn_rl_repo" not in sys.path:
    sys.path.insert(0, "/opt/trn_rl_repo")

import numpy as np

import concourse.bass as bass
import concourse.bacc as bacc
import concourse.mybir as mybir
from concourse import tile
from concourse.bass_utils import run_bass_kernel_spmd

B, S, L, D = 64, 512, 6, 512
N_CORES = 8
B_PER = B // N_CORES       # 8 batches per core
ROWS = B_PER * S           # 4096 rows per core
P = 128                    # SBUF partitions
N_BLOCKS = ROWS // P       # 32 row blocks per core
S_BLOCKS = S // P          # 4 position blocks (Ws varies with position)
KP = 2                     # row blocks per load group (512 KB per DMA)

FP32 = mybir.dt.float32
AF = mybir.ActivationFunctionType
ALU = mybir.AluOpType
AX = mybir.AxisListType

N_DVE_SUMS = 2  # how many of the L row-sums run on VectorE instead of ScalarE


def build_module(reps: int = 1) -> bass.Bass:
    """Build the kernel module.  reps>1 unrolls the whole schedule reps
    times back-to-back (identical work, same outputs) — used only for
    steady-state hardware timing: (T(reps) - T(1)) / (reps - 1)."""
    nc = bacc.Bacc("TRN2", debug=False, num_devices=N_CORES)
    xs = [
        nc.dram_tensor(f"x{j}", [ROWS, D], FP32, kind="ExternalInput").ap()
        for j in range(L)
    ]
    ws = nc.dram_tensor("Ws", [S, L * L], FP32, kind="ExternalInput").ap()
    out = nc.dram_tensor("out", [ROWS, D], FP32, kind="ExternalOutput").ap()

    def group_view(ap, b0, kp):
        # [kp*P, D] DRAM slice -> [P, kp, D] (partition, block, feature)
        return ap[b0 * P : (b0 + kp) * P, :].rearrange("(k p) d -> p k d", p=P)

    # Load groups: pairs in steady state, singles for the last four blocks so
    # the end-of-stream compute drain is one block deep, not two.
    groups = [(b, KP) for b in range(0, N_BLOCKS - 4, KP)]
    groups += [(b, 1) for b in range(N_BLOCKS - 4, N_BLOCKS)]
    groups = groups * reps

    with tile.TileContext(nc) as tc:
        with (
            tc.tile_pool(name="wpool", bufs=1) as wpool,
            tc.tile_pool(name="xpool", bufs=4) as xpool,
            tc.tile_pool(name="opool", bufs=4) as opool,
            tc.tile_pool(name="accpool", bufs=4) as accpool,
            tc.tile_pool(name="small", bufs=8) as small,
            tc.tile_pool(name="trashpool", bufs=1) as trashpool,
        ):
            # Kick off group 0's big x loads first so the DMA engines ramp
            # immediately; the tiny Ws loads follow.
            first_x = []
            for j in range(L):
                xt0 = xpool.tile([P, KP, D], FP32, tag=f"x{j}", name=f"x{j}_g0")
                nc.sync.dma_start(out=xt0[:, :, :], in_=group_view(xs[j], 0, KP))
                first_x.append(xt0)

            # Per-position 6x6 matrices, resident for the whole kernel.
            # ws_tiles[blk][p, l*6+m] = Ws[blk*128+p, l, m]
            ws_tiles = []
            for blk in range(S_BLOCKS):
                wt = wpool.tile([P, L * L], FP32, tag=f"ws{blk}")
                nc.sync.dma_start(out=wt[:, :], in_=ws[blk * P : (blk + 1) * P, :])
                ws_tiles.append(wt)

            # Garbage destinations for the row-sum accum trick (never read).
            # Separate tiles per engine so ACT/DVE don't serialize on WAW.
            trash = trashpool.tile([P, D], FP32)
            trash_dve = trashpool.tile([P, D], FP32)

            class Blk:
                __slots__ = ("x", "s", "logits", "e", "sum_e", "b", "uid")

            def stage1(st: Blk):
                # Row sums: s[p, j] = sum_d x_j[p, d]
                st.s = small.tile([P, L], FP32, tag="s", name=f"s_{st.uid}")
                for j in range(L - N_DVE_SUMS):
                    nc.scalar.activation(
                        trash[:, :], st.x[j], AF.Copy,
                        accum_out=st.s[:, j : j + 1],
                    )
                for j in range(L - N_DVE_SUMS, L):
                    nc.vector.tensor_scalar(
                        out=trash_dve[:, :], in0=st.x[j],
                        scalar1=1.0, scalar2=0.0,
                        op0=ALU.mult, op1=ALU.add,
                        accum_out=st.s[:, j : j + 1],
                    )

            def stage23(st: Blk):
                # logits[p, m] = sum_l s[p, l] * Ws[pos(p), l, m], then exp.
                # No max-subtraction: |logits| < ~20 here, exp is fp32-safe.
                blk = st.b % S_BLOCKS
                prod = small.tile([P, L * L], FP32, tag="prod", name=f"pr_{st.uid}")
                nc.vector.tensor_tensor(
                    out=prod[:, :].rearrange("p (l m) -> p l m", m=L),
                    in0=st.s[:, :].unsqueeze(2).broadcast_to((P, L, L)),
                    in1=ws_tiles[blk][:, :].rearrange("p (l m) -> p l m", m=L),
                    op=ALU.mult,
                )
                st.logits = small.tile([P, L], FP32, tag="lg", name=f"lg_{st.uid}")
                nc.vector.tensor_reduce(
                    out=st.logits[:, :],
                    in_=prod[:, :].rearrange("p (l m) -> p m l", m=L),
                    axis=AX.X,
                    op=ALU.add,
                )
                st.e = small.tile([P, L], FP32, tag="e", name=f"e_{st.uid}")
                st.sum_e = small.tile([P, 1], FP32, tag="se", name=f"se_{st.uid}")
                nc.scalar.activation(
                    st.e[:, :], st.logits[:, :], AF.Exp,
                    accum_out=st.sum_e[:, 0:1],
                )

            def stage4(st: Blk):
                # a = e / sum_e ; out[p, d] = sum_j a[p, j] * x_j[p, d]
                recip = small.tile([P, 1], FP32, tag="rc", name=f"rc_{st.uid}")
                nc.vector.reciprocal(recip[:, :], st.sum_e[:, :])
                a_t = small.tile([P, L], FP32, tag="a", name=f"a_{st.uid}")
                nc.vector.tensor_scalar_mul(a_t[:, :], st.e[:, :], recip[:, 0:1])

                o_t = opool.tile([P, D], FP32, tag="o", name=f"o_{st.uid}")
                acc = accpool.tile([P, D], FP32, tag="acc", name=f"ac_{st.uid}")
                nc.vector.tensor_scalar_mul(acc[:, :], st.x[0], a_t[:, 0:1])
                bufs = [acc[:, :], o_t[:, :]]
                for j in range(1, L):
                    nc.vector.scalar_tensor_tensor(
                        out=bufs[j % 2],
                        in0=st.x[j],
                        scalar=a_t[:, j : j + 1],
                        in1=bufs[(j + 1) % 2],
                        op0=ALU.mult,
                        op1=ALU.add,
                    )
                # L-1 = 5 steps -> result lands in o_t; store on the ACT ring.
                nc.scalar.dma_start(
                    out=out[st.b * P : (st.b + 1) * P, :], in_=o_t[:, :]
                )

            # Two-deep software pipeline over blocks; collapses to zero-deep
            # for the last blocks so the post-load drain is as short as
            # possible.
            pending: list[Blk] = []
            n_total_blocks = sum(kp for _, kp in groups)
            done23 = set()
            done4 = set()

            def emit23(i):
                if 0 <= i < len(pending) and i not in done23:
                    done23.add(i)
                    stage23(pending[i])

            def emit4(i):
                if 0 <= i < len(pending) and i not in done4 and i in done23:
                    done4.add(i)
                    stage4(pending[i])

            def tick():
                i = len(pending) - 1
                if i >= n_total_blocks - 2:
                    # tail: catch up fully, run the newest block immediately
                    for k2 in range(len(pending)):
                        emit23(k2)
                        emit4(k2)
                else:
                    emit23(i - 1)
                    emit4(i - 2)

            for gi, (b0, kp) in enumerate(groups):
                if gi == 0:
                    xg = first_x
                else:
                    xg = []
                    for j in range(L):
                        xt = xpool.tile(
                            [P, kp, D], FP32, tag=f"x{j}", name=f"x{j}_g{gi}"
                        )
                        nc.sync.dma_start(
                            out=xt[:, :, :], in_=group_view(xs[j], b0, kp)
                        )
                        xg.append(xt)
                for k in range(kp):
                    st = Blk()
                    st.b = b0 + k
                    st.uid = len(pending)
                    st.x = [xg[j][:, k, :] for j in range(L)]
                    pending.append(st)
                    stage1(st)
                    tick()
            # flush anything not yet emitted
            for k2 in range(len(pending)):
                emit23(k2)
                emit4(k2)

    # Legalize for TRN2 (≤1 sync wait per instruction) + register alloc.
    nc.compile()
    return nc


_MODULE_CACHE: bass.Bass | None = None


def _get_module() -> bass.Bass:
    global _MODULE_CACHE
    if _MODULE_CACHE is None:
        _MODULE_CACHE = build_module()
    return _MODULE_CACHE


def make_in_maps(inputs: dict) -> list:
    ws = np.ascontiguousarray(np.asarray(inputs["Ws"], dtype=np.float32)).reshape(
        S, L * L
    )
    in_maps = []
    for c in range(N_CORES):
        m = {
            f"x{j}": np.ascontiguousarray(
                np.asarray(inputs[f"x{j}"], dtype=np.float32)[
                    c * B_PER : (c + 1) * B_PER
                ]
            ).reshape(ROWS, D)
            for j in range(L)
        }
        m["Ws"] = ws
        in_maps.append(m)
    return in_maps


def kernel(**inputs) -> np.ndarray:
    nc = _get_module()
    in_maps = make_in_maps(inputs)
    res = run_bass_kernel_spmd(nc, in_maps, core_ids=list(range(N_CORES)))
    outs = [res.results[c]["out"].reshape(B_PER, S, D) for c in range(N_CORES)]
    return np.concatenate(outs, axis=0)


# revision 25
# speedup vs baseline: 44.1215x; 1.1470x over previous
"""Trainium2 Bass kernel for nn_DenselyCnnAttLayer.

Reference computation (B=64, S=512, L=6, D=512):
    X = stack([x0..x5], axis=2)                  # [B,S,L,D]
    s = X.sum(-1)                                # [B,S,L]
    logits = einsum('bsl,slm->bsm', s, Ws)       # [B,S,L]
    a = softmax(logits, -1)
    out = einsum('bsl,bsld->bsd', a, X)          # [B,S,D]

Strategy: data-parallel over batch across 8 cores (8 batches/core).
Per core the 4096 (b,s) rows are processed as 32 blocks of 128
partitions with D=512 on the free dim; blocks are loaded in pairs so
each input DMA moves 512 KB.  Row sums split between ScalarE
(activation-with-accum) and VectorE (tensor_scalar-with-accum); softmax
and the weighted accumulation (scalar_tensor_tensor chain) run on
VectorE.  Block stages are software-pipelined two deep — S1(i) row
sums, S2/S3(i-1) logits+exp, S4(i-2) weighted chain + store — so the
in-order engine streams never stall on each other's latest results.
Loads ride the SP HWDGE ring, stores the ACT ring, so store triggers
can't head-of-line block load triggers.
"""

import os
import sys

for _p in ("/opt/trn_rl_repo", "/root/.axon_site/_ro/trn_rl_repo"):
    if os.path.isdir(_p) and _p not in sys.path:
        sys.path.insert(0, _p)
        break

import numpy as np

import concourse.bass as bass
import concourse.bacc as bacc
import concourse.mybir as mybir
from concourse import tile
from concourse.bass_utils import run_bass_kernel_spmd

B, S, L, D = 64, 512, 6, 512
N_CORES = 8
B_PER = B // N_CORES       # 8 batches per core
ROWS = B_PER * S           # 4096 rows per core
P = 128                    # SBUF partitions
N_BLOCKS = ROWS // P       # 32 row blocks per core
S_BLOCKS = S // P          # 4 position blocks (Ws varies with position)
KP = 2                     # row blocks per load group (512 KB per DMA)

FP32 = mybir.dt.float32
AF = mybir.ActivationFunctionType
ALU = mybir.AluOpType
AX = mybir.AxisListType

N_DVE_SUMS = 2  # how many of the L row-sums run on VectorE instead of ScalarE


def build_module(reps: int = 1) -> bass.Bass:
    """Build the kernel module.  reps>1 unrolls the whole schedule reps
    times back-to-back (identical work, same outputs) — used only for
    steady-state hardware timing: (T(reps) - T(1)) / (reps - 1)."""
    nc = bacc.Bacc("TRN2", debug=False, num_devices=N_CORES)
    xs = [
        nc.dram_tensor(f"x{j}", [ROWS, D], FP32, kind="ExternalInput").ap()
        for j in range(L)
    ]
    ws = nc.dram_tensor("Ws", [S, L * L], FP32, kind="ExternalInput").ap()
    out = nc.dram_tensor("out", [ROWS, D], FP32, kind="ExternalOutput").ap()

    def group_view(ap, b0, kp):
        # [kp*P, D] DRAM slice -> [P, kp, D] (partition, block, feature)
        return ap[b0 * P : (b0 + kp) * P, :].rearrange("(k p) d -> p k d", p=P)

    # Load groups: pairs in steady state, singles for the last four blocks so
    # the end-of-stream compute drain is one block deep, not two.
    groups = [(b, KP) for b in range(0, N_BLOCKS - 4, KP)]
    groups += [(b, 1) for b in range(N_BLOCKS - 4, N_BLOCKS)]
    groups = groups * reps

    with tile.TileContext(nc) as tc:
        with (
            tc.tile_pool(name="wpool", bufs=1) as wpool,
            tc.tile_pool(name="xpool", bufs=4) as xpool,
            tc.tile_pool(name="opool", bufs=4) as opool,
            tc.tile_pool(name="accpool", bufs=4) as accpool,
            tc.tile_pool(name="small", bufs=8) as small,
            tc.tile_pool(name="trashpool", bufs=1) as trashpool,
        ):
            # Kick off group 0's big x loads first so the DMA engines ramp
            # immediately; the tiny Ws loads follow.
            first_x = []
            for j in range(L):
                xt0 = xpool.tile([P, KP, D], FP32, tag=f"x{j}", name=f"x{j}_g0")
                nc.sync.dma_start(out=xt0[:, :, :], in_=group_view(xs[j], 0, KP))
                first_x.append(xt0)

            # Per-position 6x6 matrices, resident for the whole kernel.
            # ws_tiles[blk][p, l*6+m] = Ws[blk*128+p, l, m]
            ws_tiles = []
            for blk in range(S_BLOCKS):
                wt = wpool.tile([P, L * L], FP32, tag=f"ws{blk}")
                nc.sync.dma_start(out=wt[:, :], in_=ws[blk * P : (blk + 1) * P, :])
                ws_tiles.append(wt)

            # Garbage destinations for the row-sum accum trick (never read).
            # Separate tiles per engine so ACT/DVE don't serialize on WAW.
            trash = trashpool.tile([P, D], FP32)
            trash_dve = trashpool.tile([P, D], FP32)

            class Blk:
                __slots__ = ("x", "s", "logits", "e", "sum_e", "b", "uid")

            def stage1(st: Blk):
                # Row sums: s[p, j] = sum_d x_j[p, d]
                st.s = small.tile([P, L], FP32, tag="s", name=f"s_{st.uid}")
                for j in range(L - N_DVE_SUMS):
                    nc.scalar.activation(
                        trash[:, :], st.x[j], AF.Copy,
                        accum_out=st.s[:, j : j + 1],
                    )
                for j in range(L - N_DVE_SUMS, L):
                    nc.vector.tensor_scalar(
                        out=trash_dve[:, :], in0=st.x[j],
                        scalar1=1.0, scalar2=0.0,
                        op0=ALU.mult, op1=ALU.add,
                        accum_out=st.s[:, j : j + 1],
                    )

            def stage23(st: Blk):
                # logits[p, m] = sum_l s[p, l] * Ws[pos(p), l, m], then exp.
                # No max-subtraction: |logits| < ~20 here, exp is fp32-safe.
                blk = st.b % S_BLOCKS
                prod = small.tile([P, L * L], FP32, tag="prod", name=f"pr_{st.uid}")
                nc.vector.tensor_tensor(
                    out=prod[:, :].rearrange("p (l m) -> p l m", m=L),
                    in0=st.s[:, :].unsqueeze(2).broadcast_to((P, L, L)),
                    in1=ws_tiles[blk][:, :].rearrange("p (l m) -> p l m", m=L),
                    op=ALU.mult,
                )
                st.logits = small.tile([P, L], FP32, tag="lg", name=f"lg_{st.uid}")
                nc.vector.tensor_reduce(
                    out=st.logits[:, :],
                    in_=prod[:, :].rearrange("p (l m) -> p m l", m=L),
                    axis=AX.X,
                    op=ALU.add,
                )
                st.e = small.tile([P, L], FP32, tag="e", name=f"e_{st.uid}")
                st.sum_e = small.tile([P, 1], FP32, tag="se", name=f"se_{st.uid}")
                nc.scalar.activation(
                    st.e[:, :], st.logits[:, :], AF.Exp,
                    accum_out=st.sum_e[:, 0:1],
                )

            def stage4(st: Blk):
                # a = e / sum_e ; out[p, d] = sum_j a[p, j] * x_j[p, d]
                recip = small.tile([P, 1], FP32, tag="rc", name=f"rc_{st.uid}")
                nc.vector.reciprocal(recip[:, :], st.sum_e[:, :])
                a_t = small.tile([P, L], FP32, tag="a", name=f"a_{st.uid}")
                nc.vector.tensor_scalar_mul(a_t[:, :], st.e[:, :], recip[:, 0:1])

                o_t = opool.tile([P, D], FP32, tag="o", name=f"o_{st.uid}")
                acc = accpool.tile([P, D], FP32, tag="acc", name=f"ac_{st.uid}")
                nc.vector.tensor_scalar_mul(acc[:, :], st.x[0], a_t[:, 0:1])
                bufs = [acc[:, :], o_t[:, :]]
                for j in range(1, L):
                    nc.vector.scalar_tensor_tensor(
                        out=bufs[j % 2],
                        in0=st.x[j],
                        scalar=a_t[:, j : j + 1],
                        in1=bufs[(j + 1) % 2],
                        op0=ALU.mult,
                        op1=ALU.add,
                    )
                # L-1 = 5 steps -> result lands in o_t; store on the ACT ring.
                nc.scalar.dma_start(
                    out=out[st.b * P : (st.b + 1) * P, :], in_=o_t[:, :]
                )

            # Two-deep software pipeline over blocks; collapses to zero-deep
            # for the last blocks so the post-load drain is as short as
            # possible.
            pending: list[Blk] = []
            n_total_blocks = sum(kp for _, kp in groups)
            done23 = set()
            done4 = set()

            def emit23(i):
                if 0 <= i < len(pending) and i not in done23:
                    done23.add(i)
                    stage23(pending[i])

            def emit4(i):
                if 0 <= i < len(pending) and i not in done4 and i in done23:
                    done4.add(i)
                    stage4(pending[i])

            def tick():
                i = len(pending) - 1
                if i >= n_total_blocks - 2:
                    # tail: catch up fully, run the newest block immediately
                    for k2 in range(len(pending)):
                        emit23(k2)
                        emit4(k2)
                else:
                    emit23(i - 1)
                    emit4(i - 2)

            for gi, (b0, kp) in enumerate(groups):
                if gi == 0:
                    xg = first_x
                else:
                    xg = []
                    for j in range(L):
                        xt = xpool.tile(
                            [P, kp, D], FP32, tag=f"x{j}", name=f"x{j}_g{gi}"
                        )
                        nc.sync.dma_start(
                            out=xt[:, :, :], in_=group_view(xs[j], b0, kp)
                        )
                        xg.append(xt)
                for k in range(kp):
                    st = Blk()
                    st.b = b0 + k
                    st.uid = len(pending)
                    st.x = [xg[j][:, k, :] for j in range(L)]
                    pending.append(st)
                    stage1(st)
                    tick()
            # flush anything not yet emitted
            for k2 in range(len(pending)):
                emit23(k2)
                emit4(k2)

    # Legalize for TRN2 (≤1 sync wait per instruction) + register alloc.
    nc.compile()
    return nc


_MODULE_CACHE: bass.Bass | None = None


def _get_module() -> bass.Bass:
    global _MODULE_CACHE
    if _MODULE_CACHE is None:
        _MODULE_CACHE = build_module()
    return _MODULE_CACHE


def make_in_maps(inputs: dict) -> list:
    ws = np.ascontiguousarray(np.asarray(inputs["Ws"], dtype=np.float32)).reshape(
        S, L * L
    )
    in_maps = []
    for c in range(N_CORES):
        m = {
            f"x{j}": np.ascontiguousarray(
                np.asarray(inputs[f"x{j}"], dtype=np.float32)[
                    c * B_PER : (c + 1) * B_PER
                ]
            ).reshape(ROWS, D)
            for j in range(L)
        }
        m["Ws"] = ws
        in_maps.append(m)
    return in_maps


def kernel(**inputs) -> np.ndarray:
    nc = _get_module()
    in_maps = make_in_maps(inputs)
    res = run_bass_kernel_spmd(nc, in_maps, core_ids=list(range(N_CORES)))
    outs = [res.results[c]["out"].reshape(B_PER, S, D) for c in range(N_CORES)]
    return np.concatenate(outs, axis=0)
